# revision 16
# baseline (speedup 1.0000x reference)
"""Trainium2 Bass kernel for AbstractGenerativeUpsample (generative sparse
upsample + existence classification + target scatter + pruning mask).

Computation (per reference):
    fea_up = einsum('nc,kcd->nkd', fea, W_up).reshape(N*K, C) + b_up
    exist  = fea_up @ W_cls + b_cls                  # [N*K, 1]
    keep   = (exist > 0) | target                    # target from scatter
    out    = where(keep, fea_up, 0)

Sharding: data-parallel over parent voxels across 8 NeuronCores.

Device strategy per core (6250 parents => 49 sub-tiles of 128):
  - fea_up via float32r matmuls (full PE rate; operands rounded to 11-bit
    mantissa).  rhs packs 2 fanout slices per matmul ([128,512] = 1 PSUM
    bank), accumulation over the 2 chunks of C_IN=256.
  - exist via a single fp32 matmul against V = W_up @ W_cls (precomputed on
    host in float64), free dim 8 -> negligible PE cost, full fp32 accuracy
    so keep-bit flips vs the reference are ~0.
  - keep = max(is_gt(exist_psum, -c0), target_mask) on VectorE.
  - PSUM->SBUF evacuation fused with the pruning mask: per-partition scale
    (keep column) on ScalarE activation(Copy) for half the slices and
    VectorE tensor_scalar_mul for the other half.
  - 1 MiB output stores ([128, 2048] f32).
"""

import numpy as np

import concourse.bass as bass
import concourse.tile as tile
from concourse import bacc, mybir
from concourse.alu_op_type import AluOpType
from concourse.bass_utils import run_bass_kernel_spmd

F32 = mybir.dt.float32
F32R = mybir.dt.float32r
ACT_COPY = mybir.ActivationFunctionType.Copy

N_PARENT = 50000
C_IN = 256
C_OUT = 256
FANOUT = 8
N_CORES = 8

NP_CORE = N_PARENT // N_CORES          # 6250 parents per core
NSUB = (NP_CORE + 127) // 128          # 49 sub-tiles of 128 parents
NPAD = NSUB * 128                      # 6272 padded parents per core
NQUAD = (NSUB + 3) // 4                # 13 quads (loads batched 4 subs/DMA)


def _round_fp32r(a: np.ndarray) -> np.ndarray:
    """Round-to-nearest-even to fp32r (fp32 with 11-bit mantissa)."""
    u = np.ascontiguousarray(a, dtype=np.float32).view(np.uint32)
    low = u & np.uint32(0xFFF)
    base = u & np.uint32(0xFFFFF000)
    lsb = (u >> np.uint32(12)) & np.uint32(1)
    round_up = (low > 0x800) | ((low == 0x800) & (lsb == 1))
    out = base + (round_up.astype(np.uint32) << np.uint32(12))
    return out.view(np.float32)


def _build_program(c0: float, repeats: int = 1):
    """Build + compile the per-core SPMD program.  Returns the Bacc.

    repeats>1 wraps the whole body in a hardware loop (for timing runs)."""
    nc = bacc.Bacc("TRN2", target_bir_lowering=False, debug=False,
                   num_devices=N_CORES)

    # Per-core DRAM I/O.
    feaT = nc.dram_tensor("feaT", [2, 128, NPAD], F32,
                          kind="ExternalInput").ap()
    w = nc.dram_tensor("w", [2, 128, 2048], F32R, kind="ExternalInput").ap()
    v = nc.dram_tensor("v", [128, 16], F32, kind="ExternalInput").ap()
    tmask = nc.dram_tensor("tmask", [128, NSUB * 8], F32,
                           kind="ExternalInput").ap()
    out_d = nc.dram_tensor("out", [NP_CORE, 2048], F32,
                           kind="ExternalOutput").ap()
    exist_d = nc.dram_tensor("exist", [128, NSUB * 8], F32,
                             kind="ExternalOutput").ap()

    def body(tc):
        with (
            tc.tile_pool(name="xin", bufs=10) as xin,
            tc.tile_pool(name="xr", bufs=3) as xrp,
            tc.tile_pool(name="small", bufs=6) as small,
            tc.tile_pool(name="aout", bufs=13) as aout,
            tc.tile_pool(name="pp", bufs=3, space="PSUM") as ppp,
            tc.tile_pool(name="pe", bufs=2, space="PSUM") as pep,
        ):
            for q in range(NQUAD):
                nq = min(512, NPAD - q * 512)       # parents in this quad
                # both C_IN chunks in one DMA: free layout = ch*nq + n
                xf = xin.tile([128, 1024], F32, tag="xf",
                              name="xf", padded_shape=[128, 1024])
                nc.sync.dma_start(
                    xf[:, 0:2 * nq].rearrange("p (a b) -> p a b", a=2),
                    feaT[:, :, q * 512:q * 512 + nq].rearrange(
                        "a p b -> p a b"))
                # fp32r copy for the full-rate fea_up matmuls
                xr = xrp.tile([128, 1024], F32R, tag="xr",
                              name="xr", padded_shape=[128, 1024])
                nc.vector.tensor_copy(xr[:, 0:2 * nq], xf[:, 0:2 * nq])

                for sub in range(4):
                    t = q * 4 + sub
                    if t >= NSUB:
                        break
                    rows = min(128, NP_CORE - t * 128)
                    ns = slice(sub * 128, sub * 128 + 128)
                    ns1 = slice(nq + sub * 128, nq + sub * 128 + 128)

                    # exist = fea @ V  (full fp32)
                    pe = pep.tile([128, 8], F32, tag="pe")
                    nc.tensor.matmul(pe[:], xf[:, ns], vsb[:, 0:8],
                                     start=True, stop=False)
                    nc.tensor.matmul(pe[:], xf[:, ns1], vsb[:, 8:16],
                                     start=False, stop=True)

                    # keep = (exist + c0 > 0) | target
                    keep = small.tile([128, 8], F32, tag="keep")
                    nc.vector.tensor_scalar(keep[:], pe[:], float(-c0), None,
                                            AluOpType.is_gt)
                    nc.vector.tensor_tensor(keep[:], keep[:],
                                            tmsb[:, t * 8:t * 8 + 8],
                                            AluOpType.max)
                    # exist output (+ c0), staged in SBUF, one DMA at the end
                    nc.scalar.activation(exsb[:, t * 8:t * 8 + 8], pe[:],
                                         ACT_COPY, bias=float(c0))

                    # fea_up: 8 pair-matmuls into 2x [128,1024] psum tiles
                    pp0 = ppp.tile([128, 1024], F32, tag="pp", name="pp0")
                    pp1 = ppp.tile([128, 1024], F32, tag="pp", name="pp1")
                    for ch, xs in ((0, ns), (1, ns1)):
                        for p in range(4):
                            dst = pp0 if p < 2 else pp1
                            off = (p % 2) * 512
                            nc.tensor.matmul(
                                dst[:, off:off + 512], xr[:, xs],
                                wsb[:, ch * 2048 + p * 512:
                                        ch * 2048 + (p + 1) * 512],
                                start=(ch == 0), stop=(ch == 1))

                    # prune-masked PSUM->SBUF evacuation, split DVE/ACT:
                    # DVE: one [128,1024] tensor_tensor with keep broadcast
                    # ACT: 4x [128,256] activation with per-partition scale
                    a_sb = aout.tile([128, 2048], F32, tag="a_sb")
                    keep_b = bass.AP(
                        tensor=keep.tensor, offset=keep.offset,
                        ap=[keep.ap[0], [1, 4], [0, 256]])
                    nc.vector.tensor_tensor(
                        a_sb[:, 0:1024].rearrange("p (k d) -> p k d", k=4),
                        pp0[:].rearrange("p (k d) -> p k d", k=4),
                        keep_b, AluOpType.mult)
                    for k in range(4, 8):
                        nc.scalar.activation(
                            a_sb[:, k * 256:(k + 1) * 256],
                            pp1[:, (k - 4) * 256:(k - 3) * 256], ACT_COPY,
                            scale=keep[:, k:k + 1])

                    nc.sync.dma_start(out_d[t * 128:t * 128 + rows, :],
                                      a_sb[:rows, :])

    with tile.TileContext(nc) as tc:
        with tc.tile_pool(name="singles", bufs=1) as singles:
            # preamble loads issue on the ACT HWDGE ring so the first fea
            # loads (SP ring) are not queued behind the 2 MiB weight load
            wsb = singles.tile([128, 4096], F32R, tag="wsb")
            nc.scalar.dma_start(wsb[:, 0:2048], w[0])
            nc.scalar.dma_start(wsb[:, 2048:4096], w[1])
            vsb = singles.tile([128, 16], F32, tag="vsb")
            nc.scalar.dma_start(vsb[:], v)
            tmsb = singles.tile([128, NSUB * 8], F32, tag="tmsb")
            nc.scalar.dma_start(tmsb[:], tmask)
            exsb = singles.tile([128, NSUB * 8], F32, tag="exsb")

            if repeats == 1:
                body(tc)
            else:
                with tc.For_i(0, repeats, 1):
                    body(tc)

            nc.sync.dma_start(exist_d, exsb[:])

    nc.compile()
    return nc


_CACHE: dict = {}


def _get_program(c0: float):
    key = round(float(c0), 12)
    if key not in _CACHE:
        _CACHE[key] = _build_program(c0)
    return _CACHE[key]


def _prepare_in_maps(fea, W_up, W_cls, target_bool):
    """Host-side sharding/layout.  Returns in_maps for the 8 cores."""
    fea = np.ascontiguousarray(np.asarray(fea, dtype=np.float32))
    W_up = np.asarray(W_up, dtype=np.float32)
    W_cls = np.asarray(W_cls, dtype=np.float32)

    # Weights: [k, c, d] -> [chunk, c_in_chunk, k, d], rounded to fp32r.
    w_host = _round_fp32r(
        np.ascontiguousarray(
            W_up.transpose(1, 0, 2).reshape(2, 128, FANOUT * C_OUT)))

    # V = W_up @ W_cls in float64: exist = fea @ V + c0.
    V = np.einsum("kcd,d->ck", W_up.astype(np.float64),
                  W_cls[:, 0].astype(np.float64))          # [256, 8]
    v_host = np.ascontiguousarray(
        V.astype(np.float32).reshape(2, 128, 8).transpose(1, 0, 2)
        .reshape(128, 16))

    in_maps = []
    for m in range(N_CORES):
        fs = fea[m * NP_CORE:(m + 1) * NP_CORE]
        fs_pad = np.zeros((NPAD, C_IN), np.float32)
        fs_pad[:NP_CORE] = fs
        feaT = np.ascontiguousarray(fs_pad.T.reshape(2, 128, NPAD))

        tm = target_bool[m * NP_CORE * FANOUT:(m + 1) * NP_CORE * FANOUT]
        tm_pad = np.zeros((NSUB * 128, FANOUT), np.float32)
        tm_pad[:NP_CORE] = tm.reshape(NP_CORE, FANOUT)
        tmask = np.ascontiguousarray(
            tm_pad.reshape(NSUB, 128, FANOUT).transpose(1, 0, 2)
            .reshape(128, NSUB * 8))

        in_maps.append({"feaT": feaT, "w": w_host, "v": v_host,
                        "tmask": tmask})
    return in_maps


def kernel(fea, W_up, b_up, W_cls, b_cls, target_idx):
    fea = np.asarray(fea, dtype=np.float32)
    W_up = np.asarray(W_up, dtype=np.float32)
    b_up = np.asarray(b_up, dtype=np.float32)
    W_cls = np.asarray(W_cls, dtype=np.float32)
    b_cls = np.asarray(b_cls, dtype=np.float32)
    target_idx = np.asarray(target_idx)

    n_up = N_PARENT * FANOUT
    target = np.zeros(n_up, dtype=bool)
    target[target_idx] = True

    c0 = float(b_up.astype(np.float64) @ W_cls[:, 0].astype(np.float64)
               + b_cls.astype(np.float64)[0])

    nc = _get_program(c0)
    in_maps = _prepare_in_maps(fea, W_up, W_cls, target)
    res = run_bass_kernel_spmd(nc, in_maps, core_ids=list(range(N_CORES)))

    fea_pruned = np.empty((n_up, C_OUT), np.float32)
    exist = np.empty((n_up, 1), np.float32)
    for m in range(N_CORES):
        r = res.results[m]
        lo = m * NP_CORE * FANOUT
        hi = (m + 1) * NP_CORE * FANOUT
        fea_pruned[lo:hi] = r["out"].reshape(-1, C_OUT)
        ex = (r["exist"].reshape(128, NSUB, 8).transpose(1, 0, 2)
              .reshape(-1)[:NP_CORE * FANOUT])
        exist[lo:hi, 0] = ex

    if np.any(b_up != 0.0):
        # device path omits the (all-zero in this problem) b_up; exact fixup
        keep = (exist[:, 0] > 0) | target
        fea_pruned[keep] += b_up[None, :]

    return fea_pruned, exist, target


# revision 17
# speedup vs baseline: 1.2862x; 1.2862x over previous
"""Trainium2 Bass kernel for AbstractGenerativeUpsample (generative sparse
upsample + existence classification + target scatter + pruning mask).

Computation (per reference):
    fea_up = einsum('nc,kcd->nkd', fea, W_up).reshape(N*K, C) + b_up
    exist  = fea_up @ W_cls + b_cls                  # [N*K, 1]
    keep   = (exist > 0) | target                    # target from scatter
    out    = where(keep, fea_up, 0)

Sharding: data-parallel over parent voxels across 8 NeuronCores.

Device strategy per core (6250 parents => 49 sub-tiles of 128):
  - fea_up via float32r matmuls (full PE rate; operands rounded to 11-bit
    mantissa).  rhs packs 2 fanout slices per matmul ([128,512] = 1 PSUM
    bank), accumulation over the 2 chunks of C_IN=256.
  - exist via a single fp32 matmul against V = W_up @ W_cls (precomputed on
    host in float64), free dim 8 -> negligible PE cost, full fp32 accuracy
    so keep-bit flips vs the reference are ~0.
  - keep = max(is_gt(exist_psum, -c0), target_mask) on VectorE.
  - PSUM->SBUF evacuation fused with the pruning mask: per-partition scale
    (keep column) on ScalarE activation(Copy) for half the slices and
    VectorE tensor_scalar_mul for the other half.
  - 1 MiB output stores ([128, 2048] f32).
"""

import numpy as np

import concourse.bass as bass
import concourse.tile as tile
from concourse import bacc, mybir
from concourse.alu_op_type import AluOpType
from concourse.bass_utils import run_bass_kernel_spmd

F32 = mybir.dt.float32
F32R = mybir.dt.float32r
ACT_COPY = mybir.ActivationFunctionType.Copy

N_PARENT = 50000
C_IN = 256
C_OUT = 256
FANOUT = 8
N_CORES = 8

NP_CORE = N_PARENT // N_CORES          # 6250 parents per core
NSUB = (NP_CORE + 127) // 128          # 49 sub-tiles of 128 parents
NPAD = NSUB * 128                      # 6272 padded parents per core
NQUAD = (NSUB + 3) // 4                # 13 quads (loads batched 4 subs/DMA)


def _round_fp32r(a: np.ndarray) -> np.ndarray:
    """Round-to-nearest-even to fp32r (fp32 with 11-bit mantissa)."""
    u = np.ascontiguousarray(a, dtype=np.float32).view(np.uint32)
    low = u & np.uint32(0xFFF)
    base = u & np.uint32(0xFFFFF000)
    lsb = (u >> np.uint32(12)) & np.uint32(1)
    round_up = (low > 0x800) | ((low == 0x800) & (lsb == 1))
    out = base + (round_up.astype(np.uint32) << np.uint32(12))
    return out.view(np.float32)


def _build_program(c0: float, repeats: int = 1):
    """Build + compile the per-core SPMD program.  Returns the Bacc.

    repeats>1 wraps the whole body in a hardware loop (for timing runs)."""
    nc = bacc.Bacc("TRN2", target_bir_lowering=False, debug=False,
                   num_devices=N_CORES)

    # Per-core DRAM I/O.
    feaT = nc.dram_tensor("feaT", [2, 128, NPAD], F32,
                          kind="ExternalInput").ap()
    w = nc.dram_tensor("w", [2, 128, 2048], F32R, kind="ExternalInput").ap()
    v = nc.dram_tensor("v", [128, 16], F32, kind="ExternalInput").ap()
    tmask = nc.dram_tensor("tmask", [128, NSUB * 8], F32,
                           kind="ExternalInput").ap()
    out_d = nc.dram_tensor("out", [NP_CORE, 2048], F32,
                           kind="ExternalOutput").ap()
    exist_d = nc.dram_tensor("exist", [128, NSUB * 8], F32,
                             kind="ExternalOutput").ap()

    def body(tc):
        with (
            tc.tile_pool(name="xin", bufs=8) as xin,
            tc.tile_pool(name="xr", bufs=3) as xrp,
            tc.tile_pool(name="small", bufs=6) as small,
            tc.tile_pool(name="aout", bufs=10) as aout,
            tc.tile_pool(name="pp", bufs=3, space="PSUM") as ppp,
            tc.tile_pool(name="pe", bufs=2, space="PSUM") as pep,
        ):
            for q in range(NQUAD):
                nq = min(512, NPAD - q * 512)       # parents in this quad
                # both C_IN chunks in one DMA: free layout = ch*nq + n
                xf = xin.tile([128, 1024], F32, tag="xf",
                              name="xf", padded_shape=[128, 1024])
                nc.sync.dma_start(
                    xf[:, 0:2 * nq].rearrange("p (a b) -> p a b", a=2),
                    feaT[:, :, q * 512:q * 512 + nq].rearrange(
                        "a p b -> p a b"))
                # fp32r copy for the full-rate fea_up matmuls
                xr = xrp.tile([128, 1024], F32R, tag="xr",
                              name="xr", padded_shape=[128, 1024])
                nc.vector.tensor_copy(xr[:, 0:2 * nq], xf[:, 0:2 * nq])

                for sub in range(4):
                    t = q * 4 + sub
                    if t >= NSUB:
                        break
                    rows = min(128, NP_CORE - t * 128)
                    ns = slice(sub * 128, sub * 128 + 128)
                    ns1 = slice(nq + sub * 128, nq + sub * 128 + 128)

                    # exist = fea @ V  (full fp32)
                    pe = pep.tile([128, 8], F32, tag="pe")
                    nc.tensor.matmul(pe[:], xf[:, ns], vsb[:, 0:8],
                                     start=True, stop=False)
                    nc.tensor.matmul(pe[:], xf[:, ns1], vsb[:, 8:16],
                                     start=False, stop=True)

                    # keep = (exist + c0 > 0) | target
                    keep = small.tile([128, 8], F32, tag="keep")
                    nc.vector.tensor_scalar(keep[:], pe[:], float(-c0), None,
                                            AluOpType.is_gt)
                    nc.vector.tensor_tensor(keep[:], keep[:],
                                            tmsb[:, t * 8:t * 8 + 8],
                                            AluOpType.max)
                    # exist output (+ c0), staged in SBUF, one DMA at the end
                    nc.scalar.activation(exsb[:, t * 8:t * 8 + 8], pe[:],
                                         ACT_COPY, bias=float(c0))

                    # fea_up: 8 pair-matmuls into 2x [128,1024] psum tiles
                    pp0 = ppp.tile([128, 1024], F32, tag="pp", name="pp0")
                    pp1 = ppp.tile([128, 1024], F32, tag="pp", name="pp1")
                    for ch, xs in ((0, ns), (1, ns1)):
                        for p in range(4):
                            dst = pp0 if p < 2 else pp1
                            off = (p % 2) * 512
                            nc.tensor.matmul(
                                dst[:, off:off + 512], xr[:, xs],
                                wsb[:, ch * 2048 + p * 512:
                                        ch * 2048 + (p + 1) * 512],
                                start=(ch == 0), stop=(ch == 1))

                    # prune-masked PSUM->SBUF evacuation, split DVE/ACT:
                    # DVE: one [128,1024] tensor_tensor with keep broadcast
                    # ACT: 4x [128,256] activation with per-partition scale
                    a_sb = aout.tile([128, 2048], F32, tag="a_sb")
                    keep_b = bass.AP(
                        tensor=keep.tensor, offset=keep.offset,
                        ap=[keep.ap[0], [1, 4], [0, 256]])
                    nc.vector.tensor_tensor(
                        a_sb[:, 0:1024].rearrange("p (k d) -> p k d", k=4),
                        pp0[:].rearrange("p (k d) -> p k d", k=4),
                        keep_b, AluOpType.mult)
                    for k in range(4, 8):
                        nc.scalar.activation(
                            a_sb[:, k * 256:(k + 1) * 256],
                            pp1[:, (k - 4) * 256:(k - 3) * 256], ACT_COPY,
                            scale=keep[:, k:k + 1])

                    nc.sync.dma_start(out_d[t * 128:t * 128 + rows, :],
                                      a_sb[:rows, :])

    with tile.TileContext(nc) as tc:
        with tc.tile_pool(name="singles", bufs=1) as singles:
            # preamble loads issue on the ACT HWDGE ring so the first fea
            # loads (SP ring) are not queued behind the 2 MiB weight load
            wsb = singles.tile([128, 4096], F32R, tag="wsb")
            nc.scalar.dma_start(wsb[:, 0:2048], w[0])
            nc.scalar.dma_start(wsb[:, 2048:4096], w[1])
            vsb = singles.tile([128, 16], F32, tag="vsb")
            nc.scalar.dma_start(vsb[:], v)
            tmsb = singles.tile([128, NSUB * 8], F32, tag="tmsb")
            nc.scalar.dma_start(tmsb[:], tmask)
            exsb = singles.tile([128, NSUB * 8], F32, tag="exsb")

            if repeats == 1:
                body(tc)
            else:
                with tc.For_i(0, repeats, 1):
                    body(tc)

            nc.sync.dma_start(exist_d, exsb[:])

    nc.compile()
    return nc


_CACHE: dict = {}


def _get_program(c0: float):
    key = round(float(c0), 12)
    if key not in _CACHE:
        _CACHE[key] = _build_program(c0)
    return _CACHE[key]


def _prepare_in_maps(fea, W_up, W_cls, target_bool):
    """Host-side sharding/layout.  Returns in_maps for the 8 cores."""
    fea = np.ascontiguousarray(np.asarray(fea, dtype=np.float32))
    W_up = np.asarray(W_up, dtype=np.float32)
    W_cls = np.asarray(W_cls, dtype=np.float32)

    # Weights: [k, c, d] -> [chunk, c_in_chunk, k, d], rounded to fp32r.
    w_host = _round_fp32r(
        np.ascontiguousarray(
            W_up.transpose(1, 0, 2).reshape(2, 128, FANOUT * C_OUT)))

    # V = W_up @ W_cls in float64: exist = fea @ V + c0.
    V = np.einsum("kcd,d->ck", W_up.astype(np.float64),
                  W_cls[:, 0].astype(np.float64))          # [256, 8]
    v_host = np.ascontiguousarray(
        V.astype(np.float32).reshape(2, 128, 8).transpose(1, 0, 2)
        .reshape(128, 16))

    in_maps = []
    for m in range(N_CORES):
        fs = fea[m * NP_CORE:(m + 1) * NP_CORE]
        fs_pad = np.zeros((NPAD, C_IN), np.float32)
        fs_pad[:NP_CORE] = fs
        feaT = np.ascontiguousarray(fs_pad.T.reshape(2, 128, NPAD))

        tm = target_bool[m * NP_CORE * FANOUT:(m + 1) * NP_CORE * FANOUT]
        tm_pad = np.zeros((NSUB * 128, FANOUT), np.float32)
        tm_pad[:NP_CORE] = tm.reshape(NP_CORE, FANOUT)
        tmask = np.ascontiguousarray(
            tm_pad.reshape(NSUB, 128, FANOUT).transpose(1, 0, 2)
            .reshape(128, NSUB * 8))

        in_maps.append({"feaT": feaT, "w": w_host, "v": v_host,
                        "tmask": tmask})
    return in_maps


def kernel(fea, W_up, b_up, W_cls, b_cls, target_idx):
    fea = np.asarray(fea, dtype=np.float32)
    W_up = np.asarray(W_up, dtype=np.float32)
    b_up = np.asarray(b_up, dtype=np.float32)
    W_cls = np.asarray(W_cls, dtype=np.float32)
    b_cls = np.asarray(b_cls, dtype=np.float32)
    target_idx = np.asarray(target_idx)

    n_up = N_PARENT * FANOUT
    target = np.zeros(n_up, dtype=bool)
    target[target_idx] = True

    c0 = float(b_up.astype(np.float64) @ W_cls[:, 0].astype(np.float64)
               + b_cls.astype(np.float64)[0])

    nc = _get_program(c0)
    in_maps = _prepare_in_maps(fea, W_up, W_cls, target)
    res = run_bass_kernel_spmd(nc, in_maps, core_ids=list(range(N_CORES)))

    fea_pruned = np.empty((n_up, C_OUT), np.float32)
    exist = np.empty((n_up, 1), np.float32)
    for m in range(N_CORES):
        r = res.results[m]
        lo = m * NP_CORE * FANOUT
        hi = (m + 1) * NP_CORE * FANOUT
        fea_pruned[lo:hi] = r["out"].reshape(-1, C_OUT)
        ex = (r["exist"].reshape(128, NSUB, 8).transpose(1, 0, 2)
              .reshape(-1)[:NP_CORE * FANOUT])
        exist[lo:hi, 0] = ex

    if np.any(b_up != 0.0):
        # device path omits the (all-zero in this problem) b_up; exact fixup
        keep = (exist[:, 0] > 0) | target
        fea_pruned[keep] += b_up[None, :]

    return fea_pruned, exist, target


# revision 19
# speedup vs baseline: 1.2866x; 1.0003x over previous
"""Trainium2 Bass kernel for AbstractGenerativeUpsample (generative sparse
upsample + existence classification + target scatter + pruning mask).

Computation (per reference):
    fea_up = einsum('nc,kcd->nkd', fea, W_up).reshape(N*K, C) + b_up
    exist  = fea_up @ W_cls + b_cls                  # [N*K, 1]
    keep   = (exist > 0) | target                    # target from scatter
    out    = where(keep, fea_up, 0)

Sharding: data-parallel over parent voxels across 8 NeuronCores.

Device strategy per core (6250 parents => 49 sub-tiles of 128):
  - fea_up via float32r matmuls (full PE rate; operands rounded to 11-bit
    mantissa).  rhs packs 2 fanout slices per matmul ([128,512] = 1 PSUM
    bank), accumulation over the 2 chunks of C_IN=256.
  - exist via a single fp32 matmul against V = W_up @ W_cls (precomputed on
    host in float64), free dim 8 -> negligible PE cost, full fp32 accuracy
    so keep-bit flips vs the reference are ~0.
  - keep = max(is_gt(exist_psum, -c0), target_mask) on VectorE.
  - PSUM->SBUF evacuation fused with the pruning mask: per-partition scale
    (keep column) on ScalarE activation(Copy) for half the slices and
    VectorE tensor_scalar_mul for the other half.
  - 1 MiB output stores ([128, 2048] f32).
"""

import numpy as np

import concourse.bass as bass
import concourse.tile as tile
from concourse import bacc, mybir
from concourse.alu_op_type import AluOpType
from concourse.bass_utils import run_bass_kernel_spmd

F32 = mybir.dt.float32
F32R = mybir.dt.float32r
ACT_COPY = mybir.ActivationFunctionType.Copy

N_PARENT = 50000
C_IN = 256
C_OUT = 256
FANOUT = 8
N_CORES = 8

NP_CORE = N_PARENT // N_CORES          # 6250 parents per core
NSUB = (NP_CORE + 127) // 128          # 49 sub-tiles of 128 parents
NPAD = NSUB * 128                      # 6272 padded parents per core
NQUAD = (NSUB + 3) // 4                # 13 quads (loads batched 4 subs/DMA)


def _round_fp32r(a: np.ndarray) -> np.ndarray:
    """Round-to-nearest-even to fp32r (fp32 with 11-bit mantissa)."""
    u = np.ascontiguousarray(a, dtype=np.float32).view(np.uint32)
    low = u & np.uint32(0xFFF)
    base = u & np.uint32(0xFFFFF000)
    lsb = (u >> np.uint32(12)) & np.uint32(1)
    round_up = (low > 0x800) | ((low == 0x800) & (lsb == 1))
    out = base + (round_up.astype(np.uint32) << np.uint32(12))
    return out.view(np.float32)


def _build_program(c0: float, repeats: int = 1):
    """Build + compile the per-core SPMD program.  Returns the Bacc.

    repeats>1 wraps the whole body in a hardware loop (for timing runs)."""
    nc = bacc.Bacc("TRN2", target_bir_lowering=False, debug=False,
                   num_devices=N_CORES)

    # Per-core DRAM I/O.
    feaT = nc.dram_tensor("feaT", [2, 128, NPAD], F32,
                          kind="ExternalInput").ap()
    w = nc.dram_tensor("w", [2, 128, 2048], F32R, kind="ExternalInput").ap()
    v = nc.dram_tensor("v", [128, 16], F32, kind="ExternalInput").ap()
    tmask = nc.dram_tensor("tmask", [128, NSUB * 8], F32,
                           kind="ExternalInput").ap()
    out_d = nc.dram_tensor("out", [NP_CORE, 2048], F32,
                           kind="ExternalOutput").ap()
    exist_d = nc.dram_tensor("exist", [128, NSUB * 8], F32,
                             kind="ExternalOutput").ap()

    def body(tc):
        with (
            tc.tile_pool(name="xin", bufs=8) as xin,
            tc.tile_pool(name="xr", bufs=3) as xrp,
            tc.tile_pool(name="small", bufs=6) as small,
            tc.tile_pool(name="aout", bufs=5) as aout,
            tc.tile_pool(name="pp", bufs=3, space="PSUM") as ppp,
            tc.tile_pool(name="pe", bufs=2, space="PSUM") as pep,
        ):
            for q in range(NQUAD):
                nq = min(512, NPAD - q * 512)       # parents in this quad
                # both C_IN chunks in one DMA: free layout = ch*nq + n
                xf = xin.tile([128, 1024], F32, tag="xf",
                              name="xf", padded_shape=[128, 1024])
                nc.sync.dma_start(
                    xf[:, 0:2 * nq].rearrange("p (a b) -> p a b", a=2),
                    feaT[:, :, q * 512:q * 512 + nq].rearrange(
                        "a p b -> p a b"))
                # fp32r copy for the full-rate fea_up matmuls
                xr = xrp.tile([128, 1024], F32R, tag="xr",
                              name="xr", padded_shape=[128, 1024])
                nc.vector.tensor_copy(xr[:, 0:2 * nq], xf[:, 0:2 * nq])

                for sub in range(4):
                    t = q * 4 + sub
                    if t >= NSUB:
                        break
                    rows = min(128, NP_CORE - t * 128)
                    ns = slice(sub * 128, sub * 128 + 128)
                    ns1 = slice(nq + sub * 128, nq + sub * 128 + 128)

                    # exist = fea @ V  (full fp32)
                    pe = pep.tile([128, 8], F32, tag="pe")
                    nc.tensor.matmul(pe[:], xf[:, ns], vsb[:, 0:8],
                                     start=True, stop=False)
                    nc.tensor.matmul(pe[:], xf[:, ns1], vsb[:, 8:16],
                                     start=False, stop=True)

                    # keep = (exist + c0 > 0) | target
                    keep = small.tile([128, 8], F32, tag="keep")
                    nc.vector.tensor_scalar(keep[:], pe[:], float(-c0), None,
                                            AluOpType.is_gt)
                    nc.vector.tensor_tensor(keep[:], keep[:],
                                            tmsb[:, t * 8:t * 8 + 8],
                                            AluOpType.max)
                    # exist output (+ c0), staged in SBUF, one DMA at the end
                    nc.scalar.activation(exsb[:, t * 8:t * 8 + 8], pe[:],
                                         ACT_COPY, bias=float(c0))

                    # fea_up: 8 pair-matmuls into 2x [128,1024] psum tiles
                    pp0 = ppp.tile([128, 1024], F32, tag="pp", name="pp0")
                    pp1 = ppp.tile([128, 1024], F32, tag="pp", name="pp1")
                    for ch, xs in ((0, ns), (1, ns1)):
                        for p in range(4):
                            dst = pp0 if p < 2 else pp1
                            off = (p % 2) * 512
                            nc.tensor.matmul(
                                dst[:, off:off + 512], xr[:, xs],
                                wsb[:, ch * 2048 + p * 512:
                                        ch * 2048 + (p + 1) * 512],
                                start=(ch == 0), stop=(ch == 1))

                    # prune-masked PSUM->SBUF evacuation, split DVE/ACT:
                    # DVE: one [128,1024] tensor_tensor with keep broadcast
                    # ACT: 4x [128,256] activation with per-partition scale
                    # Two sub-tiles share one [128,4096] staging tile so the
                    # store is a single 2 MiB DMA (dual row-block pattern).
                    if t % 2 == 0:
                        a_sb = aout.tile([128, 4096], F32, tag="a_sb",
                                         name="a_sb")
                    half = (t % 2) * 2048
                    keep_b = bass.AP(
                        tensor=keep.tensor, offset=keep.offset,
                        ap=[keep.ap[0], [1, 4], [0, 256]])
                    nc.vector.tensor_tensor(
                        a_sb[:, half:half + 1024].rearrange(
                            "p (k d) -> p k d", k=4),
                        pp0[:].rearrange("p (k d) -> p k d", k=4),
                        keep_b, AluOpType.mult)
                    for k in range(4, 8):
                        nc.scalar.activation(
                            a_sb[:, half + k * 256:half + (k + 1) * 256],
                            pp1[:, (k - 4) * 256:(k - 3) * 256], ACT_COPY,
                            scale=keep[:, k:k + 1])

                    if t == NSUB - 1:
                        # final unpaired sub-tile: single (partial) store
                        nc.sync.dma_start(
                            out_d[t * 128:t * 128 + rows, :],
                            a_sb[:rows, 0:2048])
                    elif t % 2 == 1:
                        t0 = t - 1
                        nc.sync.dma_start(
                            out_d[t0 * 128:(t0 + 2) * 128, :].rearrange(
                                "(a p) d -> p a d", a=2),
                            a_sb[:].rearrange("p (a d) -> p a d", a=2))

    with tile.TileContext(nc) as tc:
        with tc.tile_pool(name="singles", bufs=1) as singles:
            # preamble loads issue on the ACT HWDGE ring so the first fea
            # loads (SP ring) are not queued behind the 2 MiB weight load
            wsb = singles.tile([128, 4096], F32R, tag="wsb")
            nc.scalar.dma_start(wsb[:, 0:2048], w[0])
            nc.scalar.dma_start(wsb[:, 2048:4096], w[1])
            vsb = singles.tile([128, 16], F32, tag="vsb")
            nc.scalar.dma_start(vsb[:], v)
            tmsb = singles.tile([128, NSUB * 8], F32, tag="tmsb")
            nc.scalar.dma_start(tmsb[:], tmask)
            exsb = singles.tile([128, NSUB * 8], F32, tag="exsb")

            if repeats == 1:
                body(tc)
            else:
                with tc.For_i(0, repeats, 1):
                    body(tc)

            nc.sync.dma_start(exist_d, exsb[:])

    nc.compile()
    return nc


_CACHE: dict = {}


def _get_program(c0: float):
    key = round(float(c0), 12)
    if key not in _CACHE:
        _CACHE[key] = _build_program(c0)
    return _CACHE[key]


def _prepare_in_maps(fea, W_up, W_cls, target_bool):
    """Host-side sharding/layout.  Returns in_maps for the 8 cores."""
    fea = np.ascontiguousarray(np.asarray(fea, dtype=np.float32))
    W_up = np.asarray(W_up, dtype=np.float32)
    W_cls = np.asarray(W_cls, dtype=np.float32)

    # Weights: [k, c, d] -> [chunk, c_in_chunk, k, d], rounded to fp32r.
    w_host = _round_fp32r(
        np.ascontiguousarray(
            W_up.transpose(1, 0, 2).reshape(2, 128, FANOUT * C_OUT)))

    # V = W_up @ W_cls in float64: exist = fea @ V + c0.
    V = np.einsum("kcd,d->ck", W_up.astype(np.float64),
                  W_cls[:, 0].astype(np.float64))          # [256, 8]
    v_host = np.ascontiguousarray(
        V.astype(np.float32).reshape(2, 128, 8).transpose(1, 0, 2)
        .reshape(128, 16))

    in_maps = []
    for m in range(N_CORES):
        fs = fea[m * NP_CORE:(m + 1) * NP_CORE]
        fs_pad = np.zeros((NPAD, C_IN), np.float32)
        fs_pad[:NP_CORE] = fs
        feaT = np.ascontiguousarray(fs_pad.T.reshape(2, 128, NPAD))

        tm = target_bool[m * NP_CORE * FANOUT:(m + 1) * NP_CORE * FANOUT]
        tm_pad = np.zeros((NSUB * 128, FANOUT), np.float32)
        tm_pad[:NP_CORE] = tm.reshape(NP_CORE, FANOUT)
        tmask = np.ascontiguousarray(
            tm_pad.reshape(NSUB, 128, FANOUT).transpose(1, 0, 2)
            .reshape(128, NSUB * 8))

        in_maps.append({"feaT": feaT, "w": w_host, "v": v_host,
                        "tmask": tmask})
    return in_maps


def kernel(fea, W_up, b_up, W_cls, b_cls, target_idx):
    fea = np.asarray(fea, dtype=np.float32)
    W_up = np.asarray(W_up, dtype=np.float32)
    b_up = np.asarray(b_up, dtype=np.float32)
    W_cls = np.asarray(W_cls, dtype=np.float32)
    b_cls = np.asarray(b_cls, dtype=np.float32)
    target_idx = np.asarray(target_idx)

    n_up = N_PARENT * FANOUT
    target = np.zeros(n_up, dtype=bool)
    target[target_idx] = True

    c0 = float(b_up.astype(np.float64) @ W_cls[:, 0].astype(np.float64)
               + b_cls.astype(np.float64)[0])

    nc = _get_program(c0)
    in_maps = _prepare_in_maps(fea, W_up, W_cls, target)
    res = run_bass_kernel_spmd(nc, in_maps, core_ids=list(range(N_CORES)))

    fea_pruned = np.empty((n_up, C_OUT), np.float32)
    exist = np.empty((n_up, 1), np.float32)
    for m in range(N_CORES):
        r = res.results[m]
        lo = m * NP_CORE * FANOUT
        hi = (m + 1) * NP_CORE * FANOUT
        fea_pruned[lo:hi] = r["out"].reshape(-1, C_OUT)
        ex = (r["exist"].reshape(128, NSUB, 8).transpose(1, 0, 2)
              .reshape(-1)[:NP_CORE * FANOUT])
        exist[lo:hi, 0] = ex

    if np.any(b_up != 0.0):
        # device path omits the (all-zero in this problem) b_up; exact fixup
        keep = (exist[:, 0] > 0) | target
        fea_pruned[keep] += b_up[None, :]

    return fea_pruned, exist, target


# revision 21
# speedup vs baseline: 1.7990x; 1.3983x over previous
"""Trainium2 Bass kernel for AbstractGenerativeUpsample (generative sparse
upsample + existence classification + target scatter + pruning mask).

Computation (per reference):
    fea_up = einsum('nc,kcd->nkd', fea, W_up).reshape(N*K, C) + b_up
    exist  = fea_up @ W_cls + b_cls                  # [N*K, 1]
    keep   = (exist > 0) | target                    # target from scatter
    out    = where(keep, fea_up, 0)

Sharding: data-parallel over parent voxels across 8 NeuronCores.

Device strategy per core (6250 parents => 49 sub-tiles of 128):
  - fea_up via float32r matmuls (full PE rate; operands rounded to 11-bit
    mantissa).  rhs packs 2 fanout slices per matmul ([128,512] = 1 PSUM
    bank), accumulation over the 2 chunks of C_IN=256.
  - exist via a single fp32 matmul against V = W_up @ W_cls (precomputed on
    host in float64), free dim 8 -> negligible PE cost, full fp32 accuracy
    so keep-bit flips vs the reference are ~0.
  - keep = max(is_gt(exist_psum, -c0), target_mask) on VectorE.
  - PSUM->SBUF evacuation fused with the pruning mask: per-partition scale
    (keep column) on ScalarE activation(Copy) for half the slices and
    VectorE tensor_scalar_mul for the other half.
  - 1 MiB output stores ([128, 2048] f32).
"""

import numpy as np

import concourse.bass as bass
import concourse.tile as tile
from concourse import bacc, mybir
from concourse.alu_op_type import AluOpType
from concourse.bass_utils import run_bass_kernel_spmd

F32 = mybir.dt.float32
F32R = mybir.dt.float32r
ACT_COPY = mybir.ActivationFunctionType.Copy

N_PARENT = 50000
C_IN = 256
C_OUT = 256
FANOUT = 8
N_CORES = 8

NP_CORE = N_PARENT // N_CORES          # 6250 parents per core
NSUB = (NP_CORE + 127) // 128          # 49 sub-tiles of 128 parents
NPAD = NSUB * 128                      # 6272 padded parents per core
NQUAD = (NSUB + 3) // 4                # 13 quads (loads batched 4 subs/DMA)


def _round_fp32r(a: np.ndarray) -> np.ndarray:
    """Round-to-nearest-even to fp32r (fp32 with 11-bit mantissa)."""
    u = np.ascontiguousarray(a, dtype=np.float32).view(np.uint32)
    low = u & np.uint32(0xFFF)
    base = u & np.uint32(0xFFFFF000)
    lsb = (u >> np.uint32(12)) & np.uint32(1)
    round_up = (low > 0x800) | ((low == 0x800) & (lsb == 1))
    out = base + (round_up.astype(np.uint32) << np.uint32(12))
    return out.view(np.float32)


def _build_program(c0: float, repeats: int = 1):
    """Build + compile the per-core SPMD program.  Returns the Bacc.

    repeats>1 wraps the whole body in a hardware loop (for timing runs)."""
    nc = bacc.Bacc("TRN2", target_bir_lowering=False, debug=False,
                   num_devices=N_CORES)

    # Per-core DRAM I/O.
    feaT = nc.dram_tensor("feaT", [2, 128, NPAD], F32,
                          kind="ExternalInput").ap()
    w = nc.dram_tensor("w", [2, 128, 2048], F32R, kind="ExternalInput").ap()
    v = nc.dram_tensor("v", [128, 16], F32, kind="ExternalInput").ap()
    tmask = nc.dram_tensor("tmask", [128, NSUB * 8], F32,
                           kind="ExternalInput").ap()
    out_d = nc.dram_tensor("out", [NP_CORE, 2048], F32,
                           kind="ExternalOutput").ap()
    exist_d = nc.dram_tensor("exist", [128, NSUB * 8], F32,
                             kind="ExternalOutput").ap()

    def body(tc):
        with (
            tc.tile_pool(name="xin", bufs=8) as xin,
            tc.tile_pool(name="xr", bufs=3) as xrp,
            tc.tile_pool(name="small", bufs=6) as small,
            tc.tile_pool(name="aout", bufs=10) as aout,
            tc.tile_pool(name="pp", bufs=3, space="PSUM") as ppp,
            tc.tile_pool(name="pe", bufs=2, space="PSUM") as pep,
        ):
            for q in range(NQUAD):
                nq = min(512, NPAD - q * 512)       # parents in this quad
                # both C_IN chunks in one DMA: free layout = ch*nq + n
                xf = xin.tile([128, 1024], F32, tag="xf",
                              name="xf", padded_shape=[128, 1024])
                nc.sync.dma_start(
                    xf[:, 0:2 * nq].rearrange("p (a b) -> p a b", a=2),
                    feaT[:, :, q * 512:q * 512 + nq].rearrange(
                        "a p b -> p a b"))
                # fp32r copy for the full-rate fea_up matmuls
                xr = xrp.tile([128, 1024], F32R, tag="xr",
                              name="xr", padded_shape=[128, 1024])
                nc.vector.tensor_copy(xr[:, 0:2 * nq], xf[:, 0:2 * nq])

                for sub in range(4):
                    t = q * 4 + sub
                    if t >= NSUB:
                        break
                    rows = min(128, NP_CORE - t * 128)
                    ns = slice(sub * 128, sub * 128 + 128)
                    ns1 = slice(nq + sub * 128, nq + sub * 128 + 128)

                    # exist = fea @ V  (full fp32)
                    pe = pep.tile([128, 8], F32, tag="pe")
                    nc.tensor.matmul(pe[:], xf[:, ns], vsb[:, 0:8],
                                     start=True, stop=False)
                    nc.tensor.matmul(pe[:], xf[:, ns1], vsb[:, 8:16],
                                     start=False, stop=True)

                    # keep = (exist + c0 > 0) | target
                    keep = small.tile([128, 8], F32, tag="keep")
                    nc.vector.tensor_scalar(keep[:], pe[:], float(-c0), None,
                                            AluOpType.is_gt)
                    nc.vector.tensor_tensor(keep[:], keep[:],
                                            tmsb[:, t * 8:t * 8 + 8],
                                            AluOpType.max)
                    # exist output (+ c0), staged in SBUF, one DMA at the end
                    nc.scalar.activation(exsb[:, t * 8:t * 8 + 8], pe[:],
                                         ACT_COPY, bias=float(c0))

                    # fea_up: 8 pair-matmuls into 2x [128,1024] psum tiles
                    pp0 = ppp.tile([128, 1024], F32, tag="pp", name="pp0")
                    pp1 = ppp.tile([128, 1024], F32, tag="pp", name="pp1")
                    for ch, xs in ((0, ns), (1, ns1)):
                        for p in range(4):
                            dst = pp0 if p < 2 else pp1
                            off = (p % 2) * 512
                            nc.tensor.matmul(
                                dst[:, off:off + 512], xr[:, xs],
                                wsb[:, ch * 2048 + p * 512:
                                        ch * 2048 + (p + 1) * 512],
                                start=(ch == 0), stop=(ch == 1))

                    # prune-masked PSUM->SBUF evacuation, split DVE/ACT:
                    # DVE: one [128,1024] tensor_tensor with keep broadcast
                    # ACT: 4x [128,256] activation with per-partition scale
                    a_sb = aout.tile([128, 2048], F32, tag="a_sb")
                    keep_b = bass.AP(
                        tensor=keep.tensor, offset=keep.offset,
                        ap=[keep.ap[0], [1, 4], [0, 256]])
                    nc.vector.tensor_tensor(
                        a_sb[:, 0:1024].rearrange("p (k d) -> p k d", k=4),
                        pp0[:].rearrange("p (k d) -> p k d", k=4),
                        keep_b, AluOpType.mult)
                    for k in range(4, 8):
                        nc.scalar.activation(
                            a_sb[:, k * 256:(k + 1) * 256],
                            pp1[:, (k - 4) * 256:(k - 3) * 256], ACT_COPY,
                            scale=keep[:, k:k + 1])

                    nc.sync.dma_start(out_d[t * 128:t * 128 + rows, :],
                                      a_sb[:rows, :])

    with tile.TileContext(nc) as tc:
        with tc.tile_pool(name="singles", bufs=1) as singles:
            # preamble loads issue on the ACT HWDGE ring so the first fea
            # loads (SP ring) are not queued behind the 2 MiB weight load
            wsb = singles.tile([128, 4096], F32R, tag="wsb")
            nc.scalar.dma_start(wsb[:, 0:2048], w[0])
            nc.scalar.dma_start(wsb[:, 2048:4096], w[1])
            vsb = singles.tile([128, 16], F32, tag="vsb")
            nc.scalar.dma_start(vsb[:], v)
            tmsb = singles.tile([128, NSUB * 8], F32, tag="tmsb")
            nc.scalar.dma_start(tmsb[:], tmask)
            exsb = singles.tile([128, NSUB * 8], F32, tag="exsb")

            if repeats == 1:
                body(tc)
            else:
                with tc.For_i(0, repeats, 1):
                    body(tc)

            nc.sync.dma_start(exist_d, exsb[:])

    nc.compile()
    return nc


_CACHE: dict = {}


def _get_program(c0: float):
    key = round(float(c0), 12)
    if key not in _CACHE:
        _CACHE[key] = _build_program(c0)
    return _CACHE[key]


def _prepare_in_maps(fea, W_up, W_cls, target_bool):
    """Host-side sharding/layout.  Returns in_maps for the 8 cores."""
    fea = np.ascontiguousarray(np.asarray(fea, dtype=np.float32))
    W_up = np.asarray(W_up, dtype=np.float32)
    W_cls = np.asarray(W_cls, dtype=np.float32)

    # Weights: [k, c, d] -> [chunk, c_in_chunk, k, d], rounded to fp32r.
    w_host = _round_fp32r(
        np.ascontiguousarray(
            W_up.transpose(1, 0, 2).reshape(2, 128, FANOUT * C_OUT)))

    # V = W_up @ W_cls in float64: exist = fea @ V + c0.
    V = np.einsum("kcd,d->ck", W_up.astype(np.float64),
                  W_cls[:, 0].astype(np.float64))          # [256, 8]
    v_host = np.ascontiguousarray(
        V.astype(np.float32).reshape(2, 128, 8).transpose(1, 0, 2)
        .reshape(128, 16))

    in_maps = []
    for m in range(N_CORES):
        fs = fea[m * NP_CORE:(m + 1) * NP_CORE]
        fs_pad = np.zeros((NPAD, C_IN), np.float32)
        fs_pad[:NP_CORE] = fs
        feaT = np.ascontiguousarray(fs_pad.T.reshape(2, 128, NPAD))

        tm = target_bool[m * NP_CORE * FANOUT:(m + 1) * NP_CORE * FANOUT]
        tm_pad = np.zeros((NSUB * 128, FANOUT), np.float32)
        tm_pad[:NP_CORE] = tm.reshape(NP_CORE, FANOUT)
        tmask = np.ascontiguousarray(
            tm_pad.reshape(NSUB, 128, FANOUT).transpose(1, 0, 2)
            .reshape(128, NSUB * 8))

        in_maps.append({"feaT": feaT, "w": w_host, "v": v_host,
                        "tmask": tmask})
    return in_maps


def kernel(fea, W_up, b_up, W_cls, b_cls, target_idx):
    fea = np.asarray(fea, dtype=np.float32)
    W_up = np.asarray(W_up, dtype=np.float32)
    b_up = np.asarray(b_up, dtype=np.float32)
    W_cls = np.asarray(W_cls, dtype=np.float32)
    b_cls = np.asarray(b_cls, dtype=np.float32)
    target_idx = np.asarray(target_idx)

    n_up = N_PARENT * FANOUT
    target = np.zeros(n_up, dtype=bool)
    target[target_idx] = True

    c0 = float(b_up.astype(np.float64) @ W_cls[:, 0].astype(np.float64)
               + b_cls.astype(np.float64)[0])

    nc = _get_program(c0)
    in_maps = _prepare_in_maps(fea, W_up, W_cls, target)
    res = run_bass_kernel_spmd(nc, in_maps, core_ids=list(range(N_CORES)))

    fea_pruned = np.empty((n_up, C_OUT), np.float32)
    exist = np.empty((n_up, 1), np.float32)
    for m in range(N_CORES):
        r = res.results[m]
        lo = m * NP_CORE * FANOUT
        hi = (m + 1) * NP_CORE * FANOUT
        fea_pruned[lo:hi] = r["out"].reshape(-1, C_OUT)
        ex = (r["exist"].reshape(128, NSUB, 8).transpose(1, 0, 2)
              .reshape(-1)[:NP_CORE * FANOUT])
        exist[lo:hi, 0] = ex

    if np.any(b_up != 0.0):
        # device path omits the (all-zero in this problem) b_up; exact fixup
        keep = (exist[:, 0] > 0) | target
        fea_pruned[keep] += b_up[None, :]

    return fea_pruned, exist, target
